# revision 8
# baseline (speedup 1.0000x reference)
"""Trainium2 Bass kernel for nn_CrossAtten: cross-attention
out = softmax((q Wq^T)(kv Wk^T)^T / sqrt(D)) @ (kv Wv^T) @ Wout^T + bout

Shapes (hardcoded): q,kv [4,16,2048,128] fp32; Wq,Wout [128,128]; Wkv [256,128]; bout [128].
Sharding: batch*heads (64 pairs) split 8 per NeuronCore across 8 cores (pure data parallel).

v2 design (vs v1 at ~492us/core):
  - all internal compute in bf16 (host casts q/kv/A/Wvo to bf16; verified
    end-to-end rel err ~7e-3 vs the 2e-2 gate)
  - q/kv loaded via XBAR DMA-transpose straight from DRAM (kills the SWDGE
    cast loads, 256 PE transposes/task-set and 64 DVE evacuations)
  - softmax denominators OFF the tensor engine: a bf16 add-tree over the 16
    e-tiles per i-chunk (split DVE/Pool) + 4 tiny [128x128]@[128x1] matmuls
    (was 512 full N=512 matmul streams = ~200us of PE array time)
  - exp as [128,1024] pair-activations (halves ACT instruction overhead)
  - final scale+bias fused into one scalar_tensor_tensor on the Pool engine
Weights folded host-side: A = Wq^T Wk (scores = q A kv^T), Wvo = Wv^T Wout^T
(PV matmul directly yields the projected output, pre-bias).  Softmax is
max-free (logits ~ N(0,1)).
"""
import sys

if "/opt/trn_rl_repo" not in sys.path:
    sys.path.insert(0, "/opt/trn_rl_repo")

from contextlib import ExitStack

import numpy as np
import ml_dtypes

import concourse.bacc as bacc
import concourse.tile as tile
import concourse.mybir as mybir
from concourse.bass_utils import run_bass_kernel_spmd

B, H, I, J, D = 4, 16, 2048, 2048, 128
BH = B * H
N_CORES = 8
PER_CORE = BH // N_CORES          # 8 (b,h) pairs per core
P = 128                           # partitions
JT = J // P                       # 16 j-tiles
IC = 512                          # i-chunk (columns per scores/PV matmul)
NIC = I // IC                     # 4 i-chunks
NPAIR = JT // 2                   # 8 jt-pairs per chunk
SCALE = D ** -0.5

F32 = mybir.dt.float32
BF16 = mybir.dt.bfloat16
EXP = mybir.ActivationFunctionType.Exp
MULT = mybir.AluOpType.mult
ADD = mybir.AluOpType.add

_cache = {}

# tunables
CFG = dict(sk=2, pops=2)


def _build():
    nc = bacc.Bacc(
        "TRN2",
        target_bir_lowering=False,
        debug=False,
        enable_asserts=False,
        num_devices=N_CORES,
    )

    q_d = nc.dram_tensor("q", [PER_CORE, I, D], BF16, kind="ExternalInput").ap()
    kv_d = nc.dram_tensor("kv", [PER_CORE, J, D], BF16, kind="ExternalInput").ap()
    a_d = nc.dram_tensor("A", [D, D], BF16, kind="ExternalInput").ap()
    wvo_d = nc.dram_tensor("Wvo", [D, D], BF16, kind="ExternalInput").ap()
    boutb_d = nc.dram_tensor("bout_b", [P, D], F32, kind="ExternalInput").ap()
    ident_d = nc.dram_tensor("ident", [P, P], BF16, kind="ExternalInput").ap()
    out_d = nc.dram_tensor("out", [PER_CORE, I, D], F32, kind="ExternalOutput").ap()

    with tile.TileContext(nc) as tc, ExitStack() as ctx:
        const = ctx.enter_context(tc.tile_pool(name="const", bufs=1))
        tp = ctx.enter_context(tc.tile_pool(name="tp", bufs=2))
        ep = ctx.enter_context(tc.tile_pool(name="ep", bufs=4))
        wp = ctx.enter_context(tc.tile_pool(name="wp", bufs=2))
        fin = ctx.enter_context(tc.tile_pool(name="fin", bufs=2))
        # PSUM: s2 2x2 banks + pv 2x1 + ps 1x1 + po 2x(1/4) + dn tiny = 16KB
        ps2 = ctx.enter_context(tc.tile_pool(name="ps2", bufs=2, space="PSUM"))
        psv = ctx.enter_context(tc.tile_pool(name="psv", bufs=2, space="PSUM"))
        psm = ctx.enter_context(tc.tile_pool(name="psm", bufs=1, space="PSUM"))
        pso = ctx.enter_context(tc.tile_pool(name="pso", bufs=1, space="PSUM"))

        # ---- constants ----
        a_b = const.tile([D, D], BF16, tag="a_b")
        nc.sync.dma_start(a_b[:], a_d)
        wvo_b = const.tile([D, D], BF16, tag="wvo_b")
        nc.sync.dma_start(wvo_b[:], wvo_d)
        bout_b = const.tile([P, D], F32, tag="bout_b")
        nc.sync.dma_start(bout_b[:], boutb_d)
        ident_b = const.tile([P, P], BF16, tag="ident_b")
        nc.sync.dma_start(ident_b[:], ident_d)
        ones_f = const.tile([P, 1], F32, tag="ones_f")
        nc.vector.memset(ones_f[:], 1.0)
        ones_b = const.tile([P, 1], BF16, tag="ones_b")
        nc.vector.tensor_copy(ones_b[:], ones_f[:])

        TILES = {}
        SK = CFG["sk"]
        POPS = CFG["pops"]

        def _loads(k):
            # XBAR DMA transpose straight from DRAM: [2048,128]bf16 -> [128,2048]
            kvT = tp.tile([P, J], BF16, tag="kvT", name=f"kvT_{k}")
            nc.sync.dma_start_transpose(kvT[:], kv_d[k])
            qT = tp.tile([P, I], BF16, tag="qT", name=f"qT_{k}")
            nc.sync.dma_start_transpose(qT[:], q_d[k])
            TILES[k] = {"kvT": kvT, "qT": qT}

        def _setup_steps(k):
            """uT = A^T qT (4 chunks); vproj = kv @ Wvo per j-block (16)."""
            T = TILES[k]
            T["uT"] = tp.tile([P, I], BF16, tag="uT", name=f"uT_{k}")
            T["vp"] = tp.tile([P, J], BF16, tag="vp", name=f"vp_{k}")
            steps = []

            def ut_step(c):
                pu = psm.tile([P, IC], F32, tag="ps", name=f"pu_{k}_{c}")
                nc.tensor.matmul(
                    pu[:], a_b[:], T["qT"][:, c * IC : (c + 1) * IC],
                    start=True, stop=True,
                )
                nc.vector.tensor_copy(T["uT"][:, c * IC : (c + 1) * IC], pu[:])

            def vp_step(jt):
                pm = psm.tile([P, D], F32, tag="ps", name=f"pm_{k}_{jt}")
                nc.tensor.matmul(
                    pm[:], T["kvT"][:, jt * P : (jt + 1) * P], wvo_b[:],
                    start=True, stop=True,
                )
                nc.vector.tensor_copy(T["vp"][:, jt * P : (jt + 1) * P], pm[:])

            for c in range(NIC):
                steps.append(lambda c=c: ut_step(c))
            for jt in range(JT):
                steps.append(lambda jt=jt: vp_step(jt))
            return steps

        def _main(k, interleave):
            T = TILES[k]
            kvT, uT, vp = T["kvT"], T["uT"], T["vp"]
            out_sb = fin.tile([P, I], F32, tag="out_sb", name=f"o_{k}")
            pairs = [(c, jp) for c in range(NIC) for jp in range(NPAIR)]
            S, E, PSV, W, X, Y, ER, DN, RC, PVT = {}, {}, {}, {}, {}, {}, {}, {}, {}, {}
            addq = []

            def scores(idx):
                c, jp = pairs[idx]
                sp = ps2.tile([P, 2 * IC], F32, tag="s2", name=f"s_{k}_{idx}")
                for h in range(2):
                    jt = jp * 2 + h
                    nc.tensor.matmul(
                        sp[:, h * IC : (h + 1) * IC],
                        kvT[:, jt * P : (jt + 1) * P],
                        uT[:, c * IC : (c + 1) * IC],
                        start=True, stop=True,
                    )
                S[idx] = sp

            def expp(idx):
                e_t = ep.tile([P, 2 * IC], BF16, tag="e", name=f"e_{k}_{idx}")
                nc.scalar.activation(e_t[:], S.pop(idx)[:], EXP, scale=SCALE)
                E[idx] = e_t

            def consume(idx):
                c, jp = pairs[idx]
                if jp == 0:
                    PSV[c] = psv.tile([P, IC], F32, tag="pv", name=f"pv_{k}_{c}")
                e_t = E[idx]
                for h in range(2):
                    jt = jp * 2 + h
                    nc.tensor.matmul(
                        PSV[c][:],
                        vp[:, jt * P : (jt + 1) * P],
                        e_t[:, h * IC : (h + 1) * IC],
                        start=(jt == 0), stop=(jt == JT - 1),
                    )
                # denominator add-tree over the chunk's e-tiles (bf16, DVE+Pool)
                if jp % 2 == 1:
                    m = jp // 2
                    a, b = E.pop(idx - 1), E.pop(idx)

                    def l1(c=c, m=m, a=a, b=b):
                        w = wp.tile([P, 2 * IC], BF16, tag=f"w{m % 2}",
                                    name=f"w_{k}_{c}_{m}")
                        eng = nc.gpsimd if m < 3 else nc.vector
                        eng.tensor_add(w[:], a[:], b[:])
                        W[(c, m)] = w

                    addq.append(l1)
                    if m % 2 == 1:

                        def l2(c=c, x=m // 2):
                            xt = wp.tile([P, 2 * IC], BF16, tag=f"x{x}",
                                         name=f"x_{k}_{c}_{x}")
                            nc.gpsimd.tensor_add(
                                xt[:], W.pop((c, 2 * x))[:], W.pop((c, 2 * x + 1))[:]
                            )
                            X[(c, x)] = xt

                        addq.append(l2)
                    if jp == NPAIR - 1:

                        def l3(c=c):
                            yt = wp.tile([P, 2 * IC], BF16, tag="y",
                                         name=f"y_{k}_{c}")
                            nc.vector.tensor_add(
                                yt[:], X.pop((c, 0))[:], X.pop((c, 1))[:]
                            )
                            Y[c] = yt

                        def fold(c=c):
                            er_t = wp.tile([P, IC], BF16, tag="er", name=f"er_{k}_{c}")
                            yt = Y.pop(c)
                            nc.vector.tensor_add(
                                er_t[:], yt[:, 0:IC], yt[:, IC : 2 * IC]
                            )
                            ER[c] = er_t

                        def dncol(c=c):
                            dn = psm.tile([P, NIC], F32, tag="ps", name=f"dn_{k}_{c}")
                            er_t = ER.pop(c)
                            for t in range(IC // P):
                                nc.tensor.matmul(
                                    dn[:, t : t + 1],
                                    er_t[:, t * P : (t + 1) * P],
                                    ones_b[:],
                                    start=True, stop=True,
                                )
                            DN[c] = dn

                        def recips(c=c):
                            rc = fin.tile([P, IC // P], F32, tag="rc",
                                          name=f"rc_{k}_{c}")
                            nc.vector.reciprocal(rc[:], DN.pop(c)[:])
                            RC[c] = rc

                        def pvt(c=c):
                            t_ = fin.tile([P, IC], BF16, tag="pvT", name=f"pvT_{k}_{c}")
                            nc.vector.tensor_copy(t_[:], PSV.pop(c)[:])
                            PVT[c] = t_

                        addq.extend([l3, fold, pvt, dncol, recips])
                        for t in range(IC // P):

                            def fint(c=c, t=t):
                                po = pso.tile([P, P], BF16, tag="po",
                                              name=f"po_{k}_{c}_{t}")
                                nc.tensor.transpose(
                                    po[:], PVT[c][:, t * P : (t + 1) * P], ident_b[:]
                                )
                                tg = c * (IC // P) + t
                                nc.vector.scalar_tensor_tensor(
                                    out_sb[:, tg * P : (tg + 1) * P],
                                    po[:],
                                    RC[c][:, t : t + 1],
                                    bout_b[:],
                                    op0=MULT, op1=ADD,
                                )
                                if t == IC // P - 1:
                                    del PVT[c], RC[c]

                            addq.append(fint)

            for g in range(len(pairs) + SK):
                if g < len(pairs):
                    scores(g)
                if 0 <= g - 1 < len(pairs):
                    expp(g - 1)
                if g >= SK:
                    consume(g - SK)
                    for _ in range(POPS):
                        if addq:
                            addq.pop(0)()
                if interleave:
                    interleave.pop(0)()
            while addq:
                addq.pop(0)()
            # store: partition p holds rows {tg*128 + p}, 16 runs of 512B
            nc.sync.dma_start(
                out_d[k].rearrange("(tg p) e -> p tg e", p=P),
                out_sb[:].rearrange("p (tg e) -> p tg e", tg=JT),
            )
            del TILES[k]

        # prologue: task 0 loads + full setup
        _loads(0)
        for s in _setup_steps(0):
            s()
        for k in range(PER_CORE):
            pending = []
            if k + 1 < PER_CORE:
                _loads(k + 1)
                pending = _setup_steps(k + 1)
            _main(k, pending)

    nc.compile()
    return nc


def kernel(q, kv, Wq, Wkv, Wout, bout):
    if "nc" not in _cache:
        _cache["nc"] = _build()
    nc = _cache["nc"]

    Wk = Wkv[:D].astype(np.float64)
    Wv = Wkv[D:].astype(np.float64)
    A = (Wq.astype(np.float64).T @ Wk).astype(ml_dtypes.bfloat16)
    Wvo = (Wv.T @ Wout.astype(np.float64).T).astype(ml_dtypes.bfloat16)
    bout_b = np.broadcast_to(np.asarray(bout, np.float32), (P, D)).copy()
    ident = np.eye(P, dtype=ml_dtypes.bfloat16)

    qf = np.asarray(q, np.float32).reshape(BH, I, D).astype(ml_dtypes.bfloat16)
    kvf = np.asarray(kv, np.float32).reshape(BH, J, D).astype(ml_dtypes.bfloat16)

    in_maps = []
    for c in range(N_CORES):
        sl = slice(c * PER_CORE, (c + 1) * PER_CORE)
        in_maps.append(
            {
                "q": np.ascontiguousarray(qf[sl]),
                "kv": np.ascontiguousarray(kvf[sl]),
                "A": A,
                "Wvo": Wvo,
                "bout_b": bout_b,
                "ident": ident,
            }
        )

    global _last_in_maps
    _last_in_maps = in_maps

    res = run_bass_kernel_spmd(nc, in_maps, core_ids=list(range(N_CORES)))
    out = np.concatenate([r["out"] for r in res.results], axis=0)
    return out.reshape(B, H, I, D)


_last_in_maps = None


# revision 10
# speedup vs baseline: 1.0926x; 1.0926x over previous
"""Trainium2 Bass kernel for nn_CrossAtten: cross-attention
out = softmax((q Wq^T)(kv Wk^T)^T / sqrt(D)) @ (kv Wv^T) @ Wout^T + bout

Shapes (hardcoded): q,kv [4,16,2048,128] fp32; Wq,Wout [128,128]; Wkv [256,128]; bout [128].
Sharding: batch*heads (64 pairs) split 8 per NeuronCore across 8 cores (pure data parallel).

v3 design (v1 ~492us/core, v2 ~513us):
  - all internal compute in bf16 (host casts q/kv/A/Wvo to bf16; end-to-end
    rel err ~8e-3 vs the 2e-2 gate)
  - q/kv loaded via XBAR DMA-transpose straight from DRAM (kills the SWDGE
    cast loads, 256 PE transposes and 64 DVE evacuations per task-set)
  - exp as [128,1024] pair-activations (halves ACT instruction overhead)
  - softmax denominators stay on PE (ones-matmul streams; measured cheaper
    than any DVE/Pool reduction: engines read 128 elem/cycle, PE at 2.4GHz),
    accumulated per chunk into a [1,512] psum row, transposed to columns by
    4 tiny matmuls
  - PV->output transposes via XBAR DMA-transpose (14ns/128x128-tile) instead
    of PE, freeing a PSUM bank and PE slots
  - final scale+bias fused into one scalar_tensor_tensor on the (otherwise
    idle) Pool engine, reading only SBUF (Pool cannot touch PSUM)
Weights folded host-side: A = Wq^T Wk (scores = q A kv^T), Wvo = Wv^T Wout^T
(PV matmul directly yields the projected output, pre-bias).  Softmax is
max-free (logits ~ N(0,1)).
"""
import sys

if "/opt/trn_rl_repo" not in sys.path:
    sys.path.insert(0, "/opt/trn_rl_repo")

from contextlib import ExitStack

import numpy as np
import ml_dtypes

import concourse.bacc as bacc
import concourse.tile as tile
import concourse.mybir as mybir
from concourse.bass_utils import run_bass_kernel_spmd

B, H, I, J, D = 4, 16, 2048, 2048, 128
BH = B * H
N_CORES = 8
PER_CORE = BH // N_CORES          # 8 (b,h) pairs per core
P = 128                           # partitions
JT = J // P                       # 16 j-tiles
IC = 512                          # i-chunk (columns per scores/PV matmul)
NIC = I // IC                     # 4 i-chunks
NPAIR = JT // 2                   # 8 jt-pairs per chunk
SCALE = D ** -0.5

F32 = mybir.dt.float32
BF16 = mybir.dt.bfloat16
EXP = mybir.ActivationFunctionType.Exp
MULT = mybir.AluOpType.mult
ADD = mybir.AluOpType.add

_cache = {}

# tunables
CFG = dict(sk=2, pops=3)


def _build():
    nc = bacc.Bacc(
        "TRN2",
        target_bir_lowering=False,
        debug=False,
        enable_asserts=False,
        num_devices=N_CORES,
    )

    q_d = nc.dram_tensor("q", [PER_CORE, I, D], BF16, kind="ExternalInput").ap()
    kv_d = nc.dram_tensor("kv", [PER_CORE, J, D], BF16, kind="ExternalInput").ap()
    a_d = nc.dram_tensor("A", [D, D], BF16, kind="ExternalInput").ap()
    wvo_d = nc.dram_tensor("Wvo", [D, D], BF16, kind="ExternalInput").ap()
    boutb_d = nc.dram_tensor("bout_b", [P, D], F32, kind="ExternalInput").ap()
    out_d = nc.dram_tensor("out", [PER_CORE, I, D], F32, kind="ExternalOutput").ap()

    with tile.TileContext(nc) as tc, ExitStack() as ctx:
        const = ctx.enter_context(tc.tile_pool(name="const", bufs=1))
        tp = ctx.enter_context(tc.tile_pool(name="tp", bufs=2))
        ep = ctx.enter_context(tc.tile_pool(name="ep", bufs=4))
        fin = ctx.enter_context(tc.tile_pool(name="fin", bufs=2))
        po_p = ctx.enter_context(tc.tile_pool(name="po_p", bufs=3))
        # PSUM budget (8 banks): s2 2x2 + pv 2x1 + misc 1x1 + dnr 1x1
        ps2 = ctx.enter_context(tc.tile_pool(name="ps2", bufs=2, space="PSUM"))
        psv = ctx.enter_context(tc.tile_pool(name="psv", bufs=2, space="PSUM"))
        psm = ctx.enter_context(tc.tile_pool(name="psm", bufs=1, space="PSUM"))
        psd = ctx.enter_context(tc.tile_pool(name="psd", bufs=1, space="PSUM"))

        # ---- constants ----
        a_b = const.tile([D, D], BF16, tag="a_b")
        nc.sync.dma_start(a_b[:], a_d)
        wvo_b = const.tile([D, D], BF16, tag="wvo_b")
        nc.sync.dma_start(wvo_b[:], wvo_d)
        bout_b = const.tile([P, D], F32, tag="bout_b")
        nc.sync.dma_start(bout_b[:], boutb_d)
        ones_f = const.tile([P, 1], F32, tag="ones_f")
        nc.vector.memset(ones_f[:], 1.0)
        ones_b = const.tile([P, 1], BF16, tag="ones_b")
        nc.vector.tensor_copy(ones_b[:], ones_f[:])
        one1_b = const.tile([1, 1], BF16, tag="one1_b")
        nc.vector.tensor_copy(one1_b[:], ones_f[0:1, :])

        TILES = {}
        SK = CFG["sk"]
        POPS = CFG["pops"]

        def _loads(k):
            # XBAR DMA transpose straight from DRAM: [2048,128]bf16 -> [128,2048]
            kvT = tp.tile([P, J], BF16, tag="kvT", name=f"kvT_{k}")
            nc.sync.dma_start_transpose(kvT[:], kv_d[k])
            qT = tp.tile([P, I], BF16, tag="qT", name=f"qT_{k}")
            nc.sync.dma_start_transpose(qT[:], q_d[k])
            TILES[k] = {"kvT": kvT, "qT": qT}

        def _setup_steps(k):
            """uT = A^T qT (4 chunks); vproj = kv @ Wvo per j-block (16)."""
            T = TILES[k]
            T["uT"] = tp.tile([P, I], BF16, tag="uT", name=f"uT_{k}")
            T["vp"] = tp.tile([P, J], BF16, tag="vp", name=f"vp_{k}")
            steps = []

            def ut_step(c):
                pu = psm.tile([P, IC], F32, tag="ps", name=f"pu_{k}_{c}")
                nc.tensor.matmul(
                    pu[:], a_b[:], T["qT"][:, c * IC : (c + 1) * IC],
                    start=True, stop=True,
                )
                nc.vector.tensor_copy(T["uT"][:, c * IC : (c + 1) * IC], pu[:])

            def vp_step(jt):
                pm = psm.tile([P, D], F32, tag="ps", name=f"pm_{k}_{jt}")
                nc.tensor.matmul(
                    pm[:], T["kvT"][:, jt * P : (jt + 1) * P], wvo_b[:],
                    start=True, stop=True,
                )
                nc.vector.tensor_copy(T["vp"][:, jt * P : (jt + 1) * P], pm[:])

            for c in range(NIC):
                steps.append(lambda c=c: ut_step(c))
            for jt in range(JT):
                steps.append(lambda jt=jt: vp_step(jt))
            return steps

        def _main(k, interleave):
            T = TILES[k]
            kvT, uT, vp = T["kvT"], T["uT"], T["vp"]
            out_sb = fin.tile([P, I], F32, tag="out_sb", name=f"o_{k}")
            pairs = [(c, jp) for c in range(NIC) for jp in range(NPAIR)]
            S, E, PSV, DNR, RC, PVT = {}, {}, {}, {}, {}, {}
            addq = []

            def scores(idx):
                c, jp = pairs[idx]
                sp = ps2.tile([P, 2 * IC], F32, tag="s2", name=f"s_{k}_{idx}")
                for h in range(2):
                    jt = jp * 2 + h
                    nc.tensor.matmul(
                        sp[:, h * IC : (h + 1) * IC],
                        kvT[:, jt * P : (jt + 1) * P],
                        uT[:, c * IC : (c + 1) * IC],
                        start=True, stop=True,
                    )
                S[idx] = sp

            def expp(idx):
                e_t = ep.tile([P, 2 * IC], BF16, tag="e", name=f"e_{k}_{idx}")
                nc.scalar.activation(e_t[:], S.pop(idx)[:], EXP, scale=SCALE)
                E[idx] = e_t

            def consume(idx):
                c, jp = pairs[idx]
                if jp == 0:
                    PSV[c] = psv.tile([P, IC], F32, tag="pv", name=f"pv_{k}_{c}")
                    DNR[c] = psd.tile([1, IC], F32, tag="dnr", name=f"dnr_{k}_{c}")
                e_t = E.pop(idx)
                for h in range(2):
                    jt = jp * 2 + h
                    nc.tensor.matmul(
                        PSV[c][:],
                        vp[:, jt * P : (jt + 1) * P],
                        e_t[:, h * IC : (h + 1) * IC],
                        start=(jt == 0), stop=(jt == JT - 1),
                    )
                for h in range(2):
                    jt = jp * 2 + h
                    nc.tensor.matmul(
                        DNR[c][:],
                        ones_b[:],
                        e_t[:, h * IC : (h + 1) * IC],
                        start=(jt == 0), stop=(jt == JT - 1),
                    )
                if jp == NPAIR - 1:

                    def dnsb(c=c):
                        # evacuate the denominator row (bf16 is plenty)
                        row = fin.tile([1, IC], BF16, tag="dnsb", name=f"dnsb_{k}_{c}")
                        nc.vector.tensor_copy(row[:], DNR.pop(c)[:])
                        RC[("row", c)] = row

                    def pdt(c=c):
                        # transpose [1,512] row -> [128,4] columns on PE
                        pd = psd.tile([P, IC // P], F32, tag="dnr", name=f"pd_{k}_{c}")
                        row = RC.pop(("row", c))
                        for t in range(IC // P):
                            nc.tensor.matmul(
                                pd[:, t : t + 1],
                                row[:, t * P : (t + 1) * P],
                                one1_b[:],
                                start=True, stop=True,
                            )
                        RC[("pd", c)] = pd

                    def recips(c=c):
                        rc = fin.tile([P, IC // P], F32, tag="rc", name=f"rc_{k}_{c}")
                        nc.vector.reciprocal(rc[:], RC.pop(("pd", c))[:])
                        RC[c] = rc

                    def pvt(c=c):
                        t_ = fin.tile([P, IC], BF16, tag="pvT", name=f"pvT_{k}_{c}")
                        nc.vector.tensor_copy(t_[:], PSV.pop(c)[:])
                        PVT[c] = t_

                    addq.extend([dnsb, pdt, recips, pvt])
                    for t in range(IC // P):

                        def fint(c=c, t=t):
                            po = po_p.tile([P, P], BF16, tag="po",
                                           name=f"po_{k}_{c}_{t}")
                            nc.sync.dma_start_transpose(
                                po[:], PVT[c][:, t * P : (t + 1) * P]
                            )
                            tg = c * (IC // P) + t
                            nc.vector.scalar_tensor_tensor(
                                out_sb[:, tg * P : (tg + 1) * P],
                                po[:],
                                RC[c][:, t : t + 1],
                                bout_b[:],
                                op0=MULT, op1=ADD,
                            )
                            if t == IC // P - 1:
                                del PVT[c], RC[c]

                        addq.append(fint)

            for g in range(len(pairs) + SK):
                if g < len(pairs):
                    scores(g)
                if 0 <= g - 1 < len(pairs):
                    expp(g - 1)
                if g >= SK:
                    consume(g - SK)
                    for _ in range(POPS):
                        if addq:
                            addq.pop(0)()
                if interleave:
                    interleave.pop(0)()
            while addq:
                addq.pop(0)()
            # store: partition p holds rows {tg*128 + p}, 16 runs of 512B
            nc.sync.dma_start(
                out_d[k].rearrange("(tg p) e -> p tg e", p=P),
                out_sb[:].rearrange("p (tg e) -> p tg e", tg=JT),
            )
            del TILES[k]

        # prologue: task 0 loads + full setup
        _loads(0)
        for s in _setup_steps(0):
            s()
        for k in range(PER_CORE):
            pending = []
            if k + 1 < PER_CORE:
                _loads(k + 1)
                pending = _setup_steps(k + 1)
            _main(k, pending)

    nc.compile()
    return nc


def kernel(q, kv, Wq, Wkv, Wout, bout):
    if "nc" not in _cache:
        _cache["nc"] = _build()
    nc = _cache["nc"]

    Wk = Wkv[:D].astype(np.float64)
    Wv = Wkv[D:].astype(np.float64)
    A = (Wq.astype(np.float64).T @ Wk).astype(ml_dtypes.bfloat16)
    Wvo = (Wv.T @ Wout.astype(np.float64).T).astype(ml_dtypes.bfloat16)
    bout_b = np.broadcast_to(np.asarray(bout, np.float32), (P, D)).copy()

    qf = np.asarray(q, np.float32).reshape(BH, I, D).astype(ml_dtypes.bfloat16)
    kvf = np.asarray(kv, np.float32).reshape(BH, J, D).astype(ml_dtypes.bfloat16)

    in_maps = []
    for c in range(N_CORES):
        sl = slice(c * PER_CORE, (c + 1) * PER_CORE)
        in_maps.append(
            {
                "q": np.ascontiguousarray(qf[sl]),
                "kv": np.ascontiguousarray(kvf[sl]),
                "A": A,
                "Wvo": Wvo,
                "bout_b": bout_b,
            }
        )

    global _last_in_maps
    _last_in_maps = in_maps

    res = run_bass_kernel_spmd(nc, in_maps, core_ids=list(range(N_CORES)))
    out = np.concatenate([r["out"] for r in res.results], axis=0)
    return out.reshape(B, H, I, D)


_last_in_maps = None
